# revision 35
# baseline (speedup 1.0000x reference)
"""Distributed Trainium2 (8 NeuronCores) kernel for a pre-LN transformer block.

Reference computation (B=2, T=2048, E=1024, H=16, D=64):
    h1 = LN(data); q,k,v = per-head projections; causal attention (scale E^-0.5);
    x = data + concat @ Wfc + bfc; out = x + relu(LN(x) @ W1 + b1) @ W2 + b2

Sharding (Ulysses-style, SPMD-uniform across the 8 cores):
  - rows (b,t) are sharded: core c owns rows [256c, 256c+256) of each batch
    (512 rows/core, held transposed as [E, 512], col order [b0|b1])
  - LN1 + all-head QKV projections computed on local rows, then six small
    AllToAlls (Q/K/V x batch) move to head sharding; the batch-1 trio rides
    under batch-0 attention
  - heads sharded: core c owns heads {2c, 2c+1}; full-T causal attention
    (identical work on every core); softmax denominators come free from a
    ones-column appended to V; scores stay transposed [keys, q] so no PE
    transposes are needed; 2 heads pack the 64-deep contraction via
    tile_position row groups
  - per-batch AllToAlls return attention output to row sharding (batch-0
    exchange hidden under batch-1 attention)
  - Wfc + residual + LN2 + FFN + residual computed on local rows, pipelined
    by batch half so the second concat exchange hides under batch-0 tail work
All matmuls run in bf16 (f32 PSUM accumulation); LN stats and softmax exp in
f32. LayerNorm reduces over the partition (E) axis with PE ones-matmuls and
K=1 broadcast matmuls.
"""
import numpy as np
import ml_dtypes

import concourse.bass as bass
import concourse.bacc as bacc
import concourse.tile as tile
from concourse import mybir
from concourse import bass_utils

FP32 = mybir.dt.float32
BF16 = mybir.dt.bfloat16
FP8 = mybir.dt.float8e4
AF = mybir.ActivationFunctionType
OP = mybir.AluOpType

B, T, E, H, D = 2, 2048, 1024, 16, 64
NC = 8
RPB = T // NC            # 256 rows per batch per core
ROWS = B * RPB           # 512 local rows
NE = E // 128            # 8 tiles over E
F4 = 4 * E
NF = F4 // 128           # 32 tiles over 4E
NKT = T // 128           # 16 key tiles per batch
NCH = B * T // 512       # 8 row chunks of 512 over all gathered rows
EPS = 1e-5
SCALE = float(E) ** -0.5   # exactly 1/32
RG = [list(range(NC))]

_last_result = None  # BassKernelResults from the most recent run (for test harness)


def _layernorm(nc, tc, workp, statsp, eps1, x_tiles, g_col, b_col, out_factory,
               post, psname, c0=0, ncols=ROWS):
    """LayerNorm over the E (partition) axis of 8 [128, *] f32 tiles,
    restricted to columns [c0, c0+ncols). bf16 column sums on PE; f32 stats;
    per-row scale/shift broadcast via K=1 matmuls."""
    ones128 = workp.tile([128, 1], BF16, name=f"{psname}_ones128",
                         tag="lno", bufs=2)
    nc.vector.memset(ones128[:], 1.0)
    ones1f = workp.tile([1, 128], FP32, name=f"{psname}_ones1f",
                        tag="lno1", bufs=2)
    nc.vector.memset(ones1f[:], 1.0)
    cs = slice(c0, c0 + ncols)

    with tc.tile_pool(name=psname, bufs=1, space="PSUM") as ps:
        sum_ps = ps.tile([1, ncols], FP32, name=f"{psname}_sum", tag="sum")
        ssq_ps = ps.tile([1, ncols], FP32, name=f"{psname}_ssq", tag="ssq")
        for e in range(NE):
            xb = workp.tile([128, ncols], BF16, name=f"{psname}_xb{e}",
                            tag="lnsrc", bufs=2)
            nc.vector.tensor_copy(xb[:], x_tiles[e][:, cs])
            sq = workp.tile([128, ncols], BF16, name=f"{psname}_sq{e}",
                            tag="lnsq", bufs=2)
            nc.scalar.activation(sq[:], x_tiles[e][:, cs], AF.Square)
            nc.tensor.matmul(sum_ps[:], ones128[:], xb[:],
                             start=(e == 0), stop=(e == NE - 1))
            nc.tensor.matmul(ssq_ps[:], ones128[:], sq[:],
                             start=(e == 0), stop=(e == NE - 1))
        mean = statsp.tile([1, ncols], FP32, name=f"{psname}_mean", tag="v0")
        nc.vector.tensor_scalar_mul(mean[:], sum_ps[:], 1.0 / E)
        msq = statsp.tile([1, ncols], FP32, name=f"{psname}_msq", tag="v1")
        nc.vector.tensor_mul(msq[:], mean[:], mean[:])
        var = statsp.tile([1, ncols], FP32, name=f"{psname}_var", tag="v2")
        nc.vector.scalar_tensor_tensor(var[:], ssq_ps[:], 1.0 / E, msq[:],
                                       OP.mult, OP.subtract)
        std = statsp.tile([1, ncols], FP32, name=f"{psname}_std", tag="v3")
        nc.scalar.activation(std[:], var[:], AF.Sqrt, bias=eps1[:, 0:1])
        rstd = statsp.tile([1, ncols], FP32, name=f"{psname}_rstd", tag="v4")
        nc.vector.reciprocal(rstd[:], std[:])
        nmrn = statsp.tile([1, ncols], FP32, name=f"{psname}_nmrn", tag="v5")
        nc.vector.scalar_tensor_tensor(nmrn[:], mean[:], -1.0, rstd[:],
                                       OP.mult, OP.mult)
        bA = ps.tile([128, ncols], FP32, name=f"{psname}_bA", tag="sum")
        nc.tensor.matmul(bA[:], ones1f[:], rstd[:], start=True, stop=True)
        bB = ps.tile([128, ncols], FP32, name=f"{psname}_bB", tag="ssq")
        nc.tensor.matmul(bB[:], ones1f[:], nmrn[:], start=True, stop=True)
        for e in range(NE):
            t1 = workp.tile([128, ncols], FP32, name=f"{psname}_t1_{e}",
                            tag="lnt1", bufs=2)
            nc.vector.tensor_mul(t1[:], x_tiles[e][:, cs], bA[:])
            t2 = workp.tile([128, ncols], FP32, name=f"{psname}_t2_{e}",
                            tag="lnt2", bufs=2)
            nc.vector.tensor_add(t2[:], t1[:], bB[:])
            o = out_factory(e)
            nc.scalar.activation(o[:, cs], t2[:], AF.Identity,
                                 bias=b_col(e), scale=g_col(e))
            post(e, o)


def _build():
    nc = bacc.Bacc("TRN2", target_bir_lowering=False, debug=False, num_devices=NC)

    dataT_d = nc.dram_tensor("dataT", [E, ROWS], FP32, kind="ExternalInput")
    wq_d = nc.dram_tensor("wq", [E, H * D], BF16, kind="ExternalInput")
    wk_d = nc.dram_tensor("wk", [E, H * D], BF16, kind="ExternalInput")
    wv_d = nc.dram_tensor("wv", [E, H * D], BF16, kind="ExternalInput")
    wfc_d = nc.dram_tensor("wfc", [H * D, E], BF16, kind="ExternalInput")
    w1_d = nc.dram_tensor("w1", [E, F4], BF16, kind="ExternalInput")
    w2_d = nc.dram_tensor("w2", [F4, E], BF16, kind="ExternalInput")
    mask_d = nc.dram_tensor("mask", [128, 128], BF16, kind="ExternalInput")
    g1_d = nc.dram_tensor("g1", [E], FP32, kind="ExternalInput")
    be1_d = nc.dram_tensor("be1", [E], FP32, kind="ExternalInput")
    g2_d = nc.dram_tensor("g2", [E], FP32, kind="ExternalInput")
    be2_d = nc.dram_tensor("be2", [E], FP32, kind="ExternalInput")
    bfc_d = nc.dram_tensor("bfc", [E], FP32, kind="ExternalInput")
    b1_d = nc.dram_tensor("b1", [F4], FP32, kind="ExternalInput")
    b2_d = nc.dram_tensor("b2", [E], FP32, kind="ExternalInput")
    out_d = nc.dram_tensor("outT", [E, ROWS], FP32, kind="ExternalOutput")

    with tile.TileContext(nc) as tc:
        with (
            tc.tile_pool(name="constp", bufs=1) as constp,
            tc.tile_pool(name="datap", bufs=1) as datap,
            tc.tile_pool(name="workp", bufs=4) as workp,
            tc.tile_pool(name="statsp", bufs=1) as statsp,
            tc.tile_pool(name="xhp", bufs=1) as xhp,
            tc.tile_pool(name="w1p", bufs=1) as w1p,
            tc.tile_pool(name="dramp", bufs=1, space="DRAM") as dramp,
        ):
            # ---------- constant / input loads ----------
            mask_sb = constp.tile([128, 128], BF16, name="mask_sb", tag="mask")
            nc.sync.dma_start(out=mask_sb[:], in_=mask_d[:, :])
            ones1b = constp.tile([1, 128], BF16, name="ones1b", tag="ones1b")
            nc.vector.memset(ones1b[:], 1.0)
            eps1 = constp.tile([1, 1], FP32, name="eps1", tag="eps1")
            nc.vector.memset(eps1[:], EPS)
            vecs = {}
            for nm, dd, w in (("g1", g1_d, NE), ("be1", be1_d, NE), ("g2", g2_d, NE),
                              ("be2", be2_d, NE), ("bfc", bfc_d, NE), ("b2", b2_d, NE),
                              ("b1", b1_d, NF)):
                t = constp.tile([128, w], FP32, name=f"{nm}_sb", tag=nm)
                nc.sync.dma_start(out=t[:], in_=dd.ap().rearrange("(a b) -> b a", b=128))
                vecs[nm] = t

            data_t = []
            for e in range(NE):
                dt_ = datap.tile([128, ROWS], FP32, name=f"data{e}", tag=f"data{e}")
                nc.sync.dma_start(out=dt_[:], in_=dataT_d[128 * e:128 * (e + 1), :])
                data_t.append(dt_)

            # DRAM bounce buffers for the collectives
            qb_in = [dramp.tile([NC * 128, RPB], FP8, name=f"qb_in{b}",
                                tag=f"qb_in{b}") for b in range(B)]
            qb_out = [dramp.tile([NC * 128, RPB], FP8, name=f"qb_out{b}",
                                 tag=f"qb_out{b}") for b in range(B)]
            kb_in = [dramp.tile([NC * 128, RPB], FP8, name=f"kb_in{b}",
                                tag=f"kb_in{b}") for b in range(B)]
            kb_out = [dramp.tile([NC * 128, RPB], FP8, name=f"kb_out{b}",
                                 tag=f"kb_out{b}") for b in range(B)]
            vb_in = [dramp.tile([NC * 256, 128], BF16, name=f"vb_in{b}",
                                tag=f"vb_in{b}") for b in range(B)]
            vb_out = [dramp.tile([NC * 256, 128], BF16, name=f"vb_out{b}",
                                 tag=f"vb_out{b}") for b in range(B)]
            a2a_in = [dramp.tile([NC * 128, RPB], BF16, name=f"a2a_in{b}",
                                 tag=f"a2a_in{b}") for b in range(B)]
            a2a_out = [dramp.tile([NC * 128, RPB], BF16, name=f"a2a_out{b}",
                                  tag=f"a2a_out{b}") for b in range(B)]

            # ---------- phase 1: LN1 on local rows ----------
            h1l = [None] * NE

            with (
                tc.tile_pool(name="wfcp", bufs=1) as wfcp,
                tc.tile_pool(name="ccp", bufs=1) as ccp,
            ):
                cc_t = []
                for s_ in range(NC):
                    cc_t.append(ccp.tile([128, ROWS], BF16, name=f"cc{s_}",
                                         tag=f"cc{s_}"))
                with (
                    tc.tile_pool(name="qtp", bufs=1) as qtp,
                    tc.tile_pool(name="vp", bufs=1) as vp,
                    tc.tile_pool(name="clp", bufs=1) as clp,
                ):
                    QTb = [qtp.tile([128, T], FP8, name=f"QT{b}", tag=f"QT{b}")
                           for b in range(B)]
                    KTb = [qtp.tile([128, T], FP8, name=f"KT{b}", tag=f"KT{b}")
                           for b in range(B)]
                    v_ab = [vp.tile([128, NKT * 130], BF16, name=f"v_all{b}",
                                    tag=f"v_all{b}") for b in range(B)]
                    v_tb = [[v_ab[b][:, 130 * k:130 * (k + 1)]
                             for k in range(NKT)] for b in range(B)]
                    # ones columns at 64 and 129 (softmax denominator trick)
                    for b in range(B):
                        nc.vector.memset(
                            v_ab[b][:, :].rearrange("p (r h x) -> p r h x",
                                                    r=NKT, h=2)[:, :, :, 64:65],
                            1.0)
                    concatL = clp.tile([128, B * T], BF16, name="concatL",
                                       tag="concatL")

                    # --- phases 1-3: LN1, local all-head QKV, A2A to head shards ---
                    # qkv_in shard ft rows [1536*ft, +1536): Q[512] | K[512] | V[512]
                    with (
                        tc.tile_pool(name="h1lp", bufs=1) as h1lp,
                        tc.tile_pool(name="wqkvp", bufs=1) as wqkvp,
                        tc.tile_pool(name="qklp", bufs=1) as qklp,
                    ):
                        wq_t, wk_t, wv_t = [], [], []
                        for nm, dd, lst in (("wq", wq_d, wq_t), ("wk", wk_d, wk_t),
                                            ("wv", wv_d, wv_t)):
                            for e in range(NE):
                                t = wqkvp.tile([128, H * D], BF16,
                                               name=f"{nm}t{e}", tag=f"wqkv{e}",
                                               bufs=2)
                                nc.sync.dma_start(
                                    out=t[:], in_=dd[128 * e:128 * (e + 1), :])
                                lst.append(t)

                        def h1_factory(e):
                            h1l[e] = h1lp.tile([128, ROWS], BF16, name=f"h1l{e}",
                                               tag=f"h1l{e}")
                            return h1l[e]

                        _layernorm(nc, tc, workp, statsp, eps1, data_t,
                                   lambda e: vecs["g1"][:, e:e + 1],
                                   lambda e: vecs["be1"][:, e:e + 1],
                                   h1_factory, lambda e, ap: None, "ln1")

                        # Q/K then V: 8 concurrent PSUM chains (uses the
                        # banks the LN1 pool just released)
                        qkbufs = {"q": qb_in, "k": kb_in}
                        with tc.tile_pool(name="psqkv", bufs=1,
                                          space="PSUM") as psqkv:
                            for nm, wt in (("q", wq_t), ("k", wk_t)):
                                pss = [psqkv.tile([128, ROWS], FP32,
                                                  name=f"ps{nm}_{i}",
                                                  tag=f"mm{i}", bufs=1)
                                       for i in range(NE)]
                                for e in range(NE):
                                    for ft in range(NE):
                                        nc.tensor.matmul(
                                            pss[ft][:],
                                            wt[e][:, 128 * ft:128 * (ft + 1)],
                                            h1l[e][:],
                                            start=(e == 0), stop=(e == NE - 1))
                                for ft in range(NE):
                                    lt = qklp.tile([128, ROWS], FP8,
                                                   name=f"l{nm}{ft}", tag="qklq",
                                                   bufs=3)
                                    nc.scalar.copy(lt[:], pss[ft][:])
                                    for bb in range(B):
                                        nc.sync.dma_start(
                                            out=qkbufs[nm][bb][128 * ft:
                                                              128 * (ft + 1), :],
                                            in_=lt[:, RPB * bb:RPB * (bb + 1)])
                            pss = [psqkv.tile([128, 512], FP32,
                                              name=f"psvl{i}", tag=f"mm{i}",
                                              bufs=1)
                                   for i in range(NE)]
                            for e in range(NE):
                                for i in range(NE):
                                    g, rt = divmod(i, 4)
                                    nc.tensor.matmul(
                                        pss[i][:],
                                        h1l[e][:, 128 * rt:128 * (rt + 1)],
                                        wv_t[e][:, 512 * g:512 * (g + 1)],
                                        start=(e == 0), stop=(e == NE - 1))
                            for i in range(NE):
                                g, rt = divmod(i, 4)
                                lv = qklp.tile([128, 512], BF16,
                                               name=f"lv{g}_{rt}", tag="qkl",
                                               bufs=3)
                                nc.scalar.copy(lv[:], pss[i][:])
                                bb, jj = divmod(rt, 2)
                                for j4 in range(4):
                                    ft = 4 * g + j4
                                    nc.sync.dma_start(
                                        out=vb_in[bb][256 * ft + 128 * jj:
                                                      256 * ft + 128 * (jj + 1),
                                                      :],
                                        in_=lv[:, 128 * j4:128 * (j4 + 1)])
                    for bb in range(B):
                        nc.gpsimd.collective_compute(
                            "AllToAll", OP.bypass, replica_groups=RG,
                            ins=[qb_in[bb][:, :].opt()],
                            outs=[qb_out[bb][:, :].opt()])
                        nc.gpsimd.collective_compute(
                            "AllToAll", OP.bypass, replica_groups=RG,
                            ins=[kb_in[bb][:, :].opt()],
                            outs=[kb_out[bb][:, :].opt()])
                        nc.gpsimd.collective_compute(
                            "AllToAll", OP.bypass, replica_groups=RG,
                            ins=[vb_in[bb][:, :].opt()],
                            outs=[vb_out[bb][:, :].opt()])
                        for s_ in range(NC):
                            for dst, bufo in ((QTb, qb_out), (KTb, kb_out)):
                                nc.sync.dma_start(
                                    out=dst[bb][:, RPB * s_:RPB * (s_ + 1)],
                                    in_=bufo[bb][128 * s_:128 * (s_ + 1), :])
                            for j in range(2):
                                nc.sync.dma_start(
                                    out=v_tb[bb][2 * s_ + j].rearrange(
                                        "p (h x) -> p h x", h=2)[:, :, 0:64],
                                    in_=vb_out[bb][256 * s_ + 128 * j:
                                                   256 * s_ + 128 * (j + 1),
                                                   :].rearrange(
                                        "p (h x) -> p h x", h=2))

                    # prefetch Wfc while attention runs
                    wfc_t = []
                    for s in range(NE):
                        t = wfcp.tile([128, E], BF16, name=f"wfct{s}", tag=f"wfc{s}")
                        nc.sync.dma_start(out=t[:], in_=wfc_d[128 * s:128 * (s + 1), :])
                        wfc_t.append(t)

                    # ------- phase 4: causal attention for 2 heads -------
                    with (
                        tc.tile_pool(name="pst", bufs=2, space="PSUM") as pst,
                        tc.tile_pool(name="pot", bufs=4, space="PSUM") as pot,
                    ):
                        for b in range(B):
                            for qc in range(T // 512):
                                q0 = 512 * qc
                                nk = 4 * qc + 4
                                ots = []
                                for hi in range(2):
                                    ots.append(pot.tile([65, 512], FP32,
                                                        name=f"ot{b}_{qc}_{hi}",
                                                        tag="ot"))
                                for k in range(nk):
                                    off = max(0, 128 * k - q0)
                                    st = pst.tile([128, 1024], FP32,
                                                  name=f"st{b}_{qc}_{k}",
                                                  tag="st")
                                    pexp = workp.tile(
                                        [128, 1024], BF16,
                                        name=f"pex{b}_{qc}_{k}",
                                        tag="pexp", bufs=3)
                                    for hi in range(2):
                                        hp = slice(64 * hi, 64 * (hi + 1))
                                        nc.tensor.matmul(
                                            st[:, 512 * hi + off:512 * hi + 512],
                                            KTb[b][hp, 128 * k:128 * (k + 1)],
                                            QTb[b][hp, q0 + off:q0 + 512],
                                            start=True, stop=True,
                                            tile_position=(64 * hi, 0))
                                    nc.scalar.activation(
                                        pexp[:, :].rearrange(
                                            "p (h x) -> p h x", h=2)[:, :, off:512],
                                        st[:, :].rearrange(
                                            "p (h x) -> p h x", h=2)[:, :, off:512],
                                        AF.Exp, scale=SCALE)
                                    for hi in range(2):
                                        if k >= 4 * qc:  # diagonal: causal mask
                                            nc.vector.tensor_mul(
                                                pexp[:, 512 * hi + off:
                                                     512 * hi + off + 128],
                                                pexp[:, 512 * hi + off:
                                                     512 * hi + off + 128],
                                                mask_sb[:])
                                        nc.tensor.matmul(
                                            ots[hi][:, off:512],
                                            v_tb[b][k][:, 65 * hi:65 * hi + 65],
                                            pexp[:, 512 * hi + off:
                                                 512 * hi + 512],
                                            start=(k == 0), stop=(k == nk - 1))
                                for hi in range(2):
                                    rc = statsp.tile([1, 512], FP32,
                                                     name=f"rc{b}_{qc}_{hi}",
                                                     tag="rc", bufs=2)
                                    nc.vector.reciprocal(rc[:], ots[hi][64:65, :])
                                    rbs = workp.tile([64, 512], FP32,
                                                     name=f"rbs{b}_{qc}_{hi}",
                                                     tag="rbs", bufs=2)
                                    nc.gpsimd.partition_broadcast(rbs[:], rc[:])
                                    nc.vector.tensor_mul(
                                        concatL[64 * hi:64 * (hi + 1),
                                                b * T + q0: b * T + q0 + 512],
                                        ots[hi][0:64, :], rbs[:])
                            # batch-b attention done: AllToAll it back to row
                            # sharding while the next batch computes
                            if qc == T // 512 - 1:
                                for j in range(NC):
                                    nc.sync.dma_start(
                                        out=a2a_in[b][128 * j:128 * (j + 1), :],
                                        in_=concatL[:, b * T + RPB * j:
                                                    b * T + RPB * (j + 1)])
                                nc.gpsimd.collective_compute(
                                    "AllToAll", OP.bypass, replica_groups=RG,
                                    ins=[a2a_in[b][:, :].opt()],
                                    outs=[a2a_out[b][:, :].opt()])
                                for s_ in range(NC):
                                    nc.sync.dma_start(
                                        out=cc_t[s_][:, b * RPB:(b + 1) * RPB],
                                        in_=a2a_out[b][128 * s_:128 * (s_ + 1), :])

                # ---------- phases 6-8, batch-half pipelined tail ----------
                x_t, h2_t = [], []
                for e in range(NE):
                    x_t.append(xhp.tile([128, ROWS], FP32, name=f"x{e}",
                                        tag=f"x{e}"))
                    h2_t.append(xhp.tile([128, ROWS], BF16, name=f"h2_{e}",
                                         tag=f"h2{e}"))

                def wfc_half(half, ps_pool):
                    for e in range(NE):
                        ps = ps_pool.tile([128, RPB], FP32,
                                          name=f"psx{half}_{e}", tag="mm")
                        for s_ in range(NC):
                            nc.tensor.matmul(
                                ps[:], wfc_t[s_][:, 128 * e:128 * (e + 1)],
                                cc_t[s_][:, RPB * half:RPB * (half + 1)],
                                start=(s_ == 0), stop=(s_ == NC - 1))
                        nc.vector.scalar_tensor_tensor(
                            x_t[e][:, RPB * half:RPB * (half + 1)], ps[:],
                            vecs["bfc"][:, e:e + 1],
                            data_t[e][:, RPB * half:RPB * (half + 1)],
                            OP.add, OP.add)

                NSPLIT = 8   # zT f-tiles emitted per-half to cover A2A#1

                with (
                    tc.tile_pool(name="rtp", bufs=1) as rtp,
                ):
                    r_t = []
                    for f in range(NF):
                        r_t.append(rtp.tile([128, ROWS], BF16, name=f"r{f}",
                                            tag=f"r{f}"))

                    def w1_load(f, tag="w1f", bufs=4):
                        w1f = w1p.tile([128, E], BF16, name=f"w1f{f}", tag=tag,
                                       bufs=bufs)
                        src = w1_d[:, 128 * f:128 * (f + 1)].rearrange(
                            "(a p) c -> p a c", p=128)
                        nc.sync.dma_start(
                            out=w1f[:].rearrange("p (a c) -> p a c", c=128),
                            in_=src)
                        return w1f

                    def zt_chain(f, w1f, psz, c0, ncols):
                        ps = psz.tile([128, ncols], FP32,
                                      name=f"psz{f}_{c0}", tag="mm")
                        for e in range(NE):
                            nc.tensor.matmul(
                                ps[:], w1f[:, 128 * e:128 * (e + 1)],
                                h2_t[e][:, c0:c0 + ncols],
                                start=(e == 0), stop=(e == NE - 1))
                        nc.scalar.activation(r_t[f][:, c0:c0 + ncols], ps[:],
                                             AF.Relu, bias=vecs["b1"][:, f:f + 1])

                    with tc.tile_pool(name="psfc", bufs=2, space="PSUM") as psfc:
                        wfc_half(0, psfc)
                    _layernorm(nc, tc, workp, statsp, eps1, x_t,
                               lambda e: vecs["g2"][:, e:e + 1],
                               lambda e: vecs["be2"][:, e:e + 1],
                               lambda e: h2_t[e], lambda e, ap: None, "ln2a",
                               c0=0, ncols=RPB)
                    w1fs = {}
                    with tc.tile_pool(name="psz", bufs=2, space="PSUM") as psz:
                        for f in range(NSPLIT):
                            w1fs[f] = w1_load(f, tag=f"w1k{f}", bufs=1)
                            zt_chain(f, w1fs[f], psz, 0, RPB)
                        # batch-1 catch-up (waits on the concat AllToAll)
                        with tc.tile_pool(name="psfc2", bufs=2,
                                          space="PSUM") as psfc2:
                            wfc_half(1, psfc2)
                        _layernorm(nc, tc, workp, statsp, eps1, x_t,
                                   lambda e: vecs["g2"][:, e:e + 1],
                                   lambda e: vecs["be2"][:, e:e + 1],
                                   lambda e: h2_t[e], lambda e, ap: None, "ln2b",
                                   c0=RPB, ncols=RPB)
                        for f in range(NSPLIT):
                            zt_chain(f, w1fs[f], psz, RPB, RPB)
                        for f in range(NSPLIT, NF):
                            w1f = w1_load(f)
                            zt_chain(f, w1f, psz, 0, ROWS)
                    with (
                        tc.tile_pool(name="w2p", bufs=3) as w2p,
                        tc.tile_pool(name="psff", bufs=1, space="PSUM") as psff,
                    ):
                        ff_ps = []
                        for e in range(NE):
                            ff_ps.append(psff.tile([128, ROWS], FP32,
                                                   name=f"ff{e}", tag=f"ff{e}"))
                        for f in range(NF):
                            w2t = w2p.tile([128, E], BF16, name=f"w2t{f}",
                                           tag="w2")
                            nc.sync.dma_start(out=w2t[:],
                                              in_=w2_d[128 * f:128 * (f + 1), :])
                            for e in range(NE):
                                nc.tensor.matmul(ff_ps[e][:],
                                                 w2t[:, 128 * e:128 * (e + 1)],
                                                 r_t[f][:],
                                                 start=(f == 0),
                                                 stop=(f == NF - 1))
                        for e in range(NE):
                            o = workp.tile([128, ROWS], FP32, name=f"o{e}",
                                           tag="o", bufs=2)
                            nc.vector.scalar_tensor_tensor(
                                o[:], ff_ps[e][:], vecs["b2"][:, e:e + 1],
                                x_t[e][:], OP.add, OP.add)
                            nc.sync.dma_start(out=out_d[128 * e:128 * (e + 1), :],
                                              in_=o[:])

    nc.compile()
    return nc


def _shard(inputs):
    bf = ml_dtypes.bfloat16
    data = np.asarray(inputs["data"], np.float32)
    Wq = np.asarray(inputs["Wq"], np.float32)
    Wk = np.asarray(inputs["Wk"], np.float32)
    Wv = np.asarray(inputs["Wv"], np.float32)
    wfc = np.ascontiguousarray(np.asarray(inputs["Wfc"], np.float32).astype(bf))
    w1 = np.ascontiguousarray(np.asarray(inputs["W1"], np.float32).astype(bf))
    w2 = np.ascontiguousarray(np.asarray(inputs["W2"], np.float32).astype(bf))
    kk, qq = np.meshgrid(np.arange(128), np.arange(128), indexing="ij")
    mask = np.ascontiguousarray((kk <= qq).astype(bf))
    common = dict(wfc=wfc, w1=w1, w2=w2, mask=mask)
    common["wq"] = np.ascontiguousarray(
        Wq.transpose(1, 0, 2).reshape(E, H * D).astype(bf))
    common["wk"] = np.ascontiguousarray(
        Wk.transpose(1, 0, 2).reshape(E, H * D).astype(bf))
    common["wv"] = np.ascontiguousarray(
        Wv.transpose(1, 0, 2).reshape(E, H * D).astype(bf))
    for nm in ("g1", "be1", "g2", "be2", "bfc", "b1", "b2"):
        common[nm] = np.ascontiguousarray(np.asarray(inputs[nm], np.float32))
    in_maps = []
    for c in range(NC):
        rows = np.concatenate([data[0, RPB * c:RPB * (c + 1)],
                               data[1, RPB * c:RPB * (c + 1)]], axis=0)  # [512, E]
        m = dict(common)
        m["dataT"] = np.ascontiguousarray(rows.T)
        in_maps.append(m)
    return in_maps


_nc_cache = None


def kernel(**inputs):
    global _last_result, _nc_cache
    if _nc_cache is None:
        _nc_cache = _build()
    in_maps = _shard(inputs)
    res = bass_utils.run_bass_kernel_spmd(
        _nc_cache, in_maps, core_ids=list(range(NC)))
    _last_result = res
    out = np.zeros((B, T, E), np.float32)
    for c in range(NC):
        ot = np.asarray(res.results[c]["outT"], np.float32)  # [E, 512]
        out[0, RPB * c:RPB * (c + 1)] = ot[:, 0:RPB].T
        out[1, RPB * c:RPB * (c + 1)] = ot[:, RPB:ROWS].T
    return out
